# revision 6
# baseline (speedup 1.0000x reference)
"""Bicubic 4x upsample (Keys a=-0.75) on 8 Trainium2 NeuronCores.

Strategy
--------
Data parallel over the batch: core i handles images [2i, 2i+1] (6
image-channels of 256x256 each), no cross-core communication.

Per image-channel the separable bicubic upsample is expressed as two
banded matmuls on the TensorEngine with NO transposes:

  pass A:  t1t[wi, ho] = sum_hi xp[hi, wi] * Ut[hi, ho]      (vertical)
  pass B:  out[ho, wo] = sum_wi t1t[wi, ho] * Ut[wi, wo]     (horizontal)

where xp is the edge-padded [259, 259] input (as stored: partitions =
rows = hi) and Ut [259, 1024] is the transposed upsample matrix
Ut[i+j, 4i+d] = K[d, j].  Both passes use matmul(out, lhsT, rhs) =
lhsT.T @ rhs: pass A takes lhsT = xp (already [hi, wi]), pass B takes
lhsT = t1t (already [wi, ho]).

Banding: an output column chunk [512n, 512n+512) needs contraction
rows [128n, 128n+131).  We supply it as one K=128 matmul over the full
512 columns plus one K=3 accumulating matmul over ONLY the last 12
columns -- Ut's 4-tap band means rows [128(n+1), 128(n+1)+3) touch
just columns [512(n+1)-12, 512(n+1)).  (The previous version ran the
K=3 correction over all 512 columns; PE matmul cost is driven by the
streamed column count, so that doubled TensorE time for no reason.)

Everything is float16 end to end: the host casts the padded input and
Ut to fp16 (in-flight HBM loads need no SWDGE cast), matmuls run fp16
(full PE rate, fp32 PSUM accumulation), and the output is STORED fp16
then upcast to f32 on the host.  The problem is HBM-write-bound, so
halving the 25 MB/core f32 output to 12.6 MB fp16 halves the dominant
traffic; measured scale-relative error stays ~1.5e-3 against the f64
reference (gate 2e-2; bicubic tap weights are exact in fp16).  PSUM
results are copied to SBUF alternating VectorE / ScalarE (DMA cannot
read PSUM), then DMA'd out per 128-row chunk on the sync HWDGE ring,
which carries nothing but the output stores.
"""

import os
import numpy as np

N, C, H, W = 16, 3, 256, 256
SCALE = 4
HP = H + 3                # padded rows/cols (left 1, right 2, edge mode)
HO, WO = H * SCALE, W * SCALE
NCORES = 8
IMGS_PER_CORE = N // NCORES
NIC = IMGS_PER_CORE * C   # image-channels per core

_CACHE = {}


def _build_ut(kernels: np.ndarray) -> np.ndarray:
    """Ut[hi, ho] with Ut[i+j, 4i+d] = K[d, j]; zeros off the band."""
    ut = np.zeros((HP, HO), dtype=np.float32)
    ii = np.arange(H)
    for j in range(4):
        for d in range(4):
            ut[ii + j, SCALE * ii + d] = kernels[d, j]
    return ut


def _build_nc(n_reps: int = 1, mm_dtype: str = "float16",
              out_dtype: str = "float16", in_path: str = "gpsimd",
              io_dtype: str = "float16", corr_cols: int = 12,
              ob_wide: bool = False, stagger: bool = False,
              store_split: bool = False):
    from concourse import bacc, mybir, tile

    f32 = mybir.dt.float32
    f32r = getattr(mybir.dt, mm_dtype)
    of = getattr(mybir.dt, out_dtype)
    iof = getattr(mybir.dt, io_dtype)
    assert io_dtype == mm_dtype or io_dtype == "float32"
    cast_in = io_dtype != mm_dtype

    nc = bacc.Bacc(
        "TRN2", target_bir_lowering=False, debug=False, enable_asserts=False
    )
    xp_d = nc.declare_dram_parameter("xp", [NIC, HP, HP], iof, isOutput=False)
    ut_d = nc.declare_dram_parameter("ut", [HP, HO], iof, isOutput=False)
    out_d = nc.declare_dram_parameter("out", [NIC, HO, WO], of, isOutput=True)

    # contraction row tiles: {0:128, 128:256, 256:259}
    ROWS = [(0, 128), (128, 256), (256, 259)]

    with tile.TileContext(nc) as tc:
        xin_bufs = int(os.environ.get("XIN_BUFS", "2"))
        mid_bufs = int(os.environ.get("MID_BUFS", "2"))
        ob_bufs = int(os.environ.get("OB_BUFS", "4"))
        psa_bufs = int(os.environ.get("PSA_BUFS", "2"))
        psb_bufs = int(os.environ.get("PSB_BUFS", "2"))
        with (
            tc.tile_pool(name="const", bufs=1) as cpool,
            tc.tile_pool(name="xin", bufs=xin_bufs) as xpool,
            tc.tile_pool(name="mid", bufs=mid_bufs) as mpool,
            tc.tile_pool(name="ob", bufs=ob_bufs) as opool,
            tc.tile_pool(name="psa", bufs=psa_bufs, space="PSUM") as psa,
            tc.tile_pool(name="psb", bufs=psb_bufs, space="PSUM") as psb,
        ):
            ut_t = []
            for r, (a, b) in enumerate(ROWS):
                t = cpool.tile([b - a, HO], f32r, tag=f"ut{r}", name=f"ut{r}")
                nc.gpsimd.dma_start(t[:], ut_d[a:b, :])
                ut_t.append(t)

            def mm_chunks(ps, lhs_tiles, msl):
                """Banded matmuls for all HO columns into PSUM tile ps.

                lhs_tiles[r] holds contraction rows ROWS[r]; msl slices
                the M (free) dim of each lhs tile.
                """
                for n2 in range(2):
                    c0 = 512 * n2
                    nc.tensor.matmul(
                        ps[:, c0:c0 + 512],
                        lhs_tiles[n2][:, msl],
                        ut_t[n2][:, c0:c0 + 512],
                        start=True, stop=True,
                    )
                    # 4-tap band: rows [128(n2+1), 128(n2+1)+3) only touch
                    # the last corr_cols columns of this 512-column chunk
                    cc = 512 * (n2 + 1) - corr_cols
                    nc.tensor.matmul(
                        ps[:, cc:cc + corr_cols],
                        lhs_tiles[n2 + 1][0:3, msl],
                        ut_t[n2 + 1][0:3, cc:cc + corr_cols],
                        start=False, stop=True,
                    )

            def body():
                for ic in range(NIC):
                    xq = []
                    for r, (a, b) in enumerate(ROWS):
                        t = xpool.tile(
                            [b - a, HP], f32r, tag=f"xq{r}", name=f"xq{r}_{ic}"
                        )
                        if cast_in or in_path == "gpsimd":
                            nc.gpsimd.dma_start(t[:], xp_d[ic, a:b, :])
                        else:
                            nc.sync.dma_start(t[:], xp_d[ic, a:b, :])
                        xq.append(t)

                    # ---- pass A: t1t[wi, ho], M-chunks = ROWS of wi ----
                    t1t = []
                    for m, (ma, mb) in enumerate(ROWS):
                        pa = psa.tile([mb - ma, HO], f32, tag="psa",
                                      name=f"pa{ic}_{m}")
                        mm_chunks(pa, xq, slice(ma, mb))
                        tt = mpool.tile([mb - ma, HO], f32r, tag=f"t1t{m}",
                                        name=f"t1t{m}_{ic}")
                        if m == 0:
                            nc.vector.tensor_copy(tt[:], pa[:])
                        else:
                            nc.scalar.copy(tt[:], pa[:])
                        t1t.append(tt)

                    # ---- pass B: out[ho, wo], 8 M-chunks of 128 ho rows ----
                    if ob_wide:
                        obw = opool.tile([128, 8 * WO], of, tag="obw",
                                         name=f"obw{ic}")
                    for m in range(8):
                        msl = slice(128 * m, 128 * m + 128)
                        pb = psb.tile([128, WO], f32, tag="psb",
                                      name=f"pb{ic}_{m}")
                        mm_chunks(pb, t1t, msl)
                        dst = (obw[:, m * WO:(m + 1) * WO] if ob_wide
                               else None)
                        if dst is None:
                            ob = opool.tile([128, WO], of, tag="ob",
                                            name=f"ob{ic}_{m}")
                            dst = ob[:]
                        if m % 2 == 0:
                            nc.vector.tensor_copy(dst, pb[:])
                        else:
                            nc.scalar.copy(dst, pb[:])
                        if not ob_wide:
                            st = (nc.gpsimd if store_split and m % 2 == 1
                                  else nc.sync)
                            st.dma_start(out_d[ic, msl, :], dst)
                    if ob_wide:
                        dram_v = out_d[ic].rearrange("(m p) w -> p m w", p=128)
                        sbuf_v = obw[:].rearrange("p (m w) -> p m w", m=8)
                        st = (nc.gpsimd if store_split and ic % 2 == 1
                              else nc.sync)
                        st.dma_start(dram_v, sbuf_v)

            if n_reps == 1:
                body()
            else:
                # timing mode: repeat the whole kernel body on-device so the
                # per-iteration HW time can be extracted from wall-clock delta
                with tc.For_i(0, n_reps, 1,
                              hint_engines=(mybir.EngineType.PE,),
                              staggered_reset=stagger):
                    body()

    nc.compile()
    return nc


def _cfg():
    return dict(
        mm_dtype=os.environ.get("MM_DTYPE", "float16"),
        out_dtype=os.environ.get("OUT_DTYPE", "float16"),
        in_path=os.environ.get("IN_PATH", "gpsimd"),
        io_dtype=os.environ.get("IO_DTYPE", "float16"),
        corr_cols=int(os.environ.get("CORR_COLS", "12")),
        ob_wide=os.environ.get("OB_WIDE", "0") == "1",
        stagger=os.environ.get("STAGGER", "0") == "1",
        store_split=os.environ.get("STORE_SPLIT", "0") == "1",
    )


def get_nc(n_reps: int = 1, **over):
    cfg = {**_cfg(), **over}
    key = ("nc", n_reps, *sorted(cfg.items()))
    if key not in _CACHE:
        _CACHE[key] = _build_nc(n_reps, **cfg)
    return _CACHE[key]


def _default_kernels():
    # deterministic Keys a=-0.75 taps, matching the module under test
    A = -0.75
    cubic = np.array(
        [[0.0, A, -2.0 * A, A],
         [1.0, 0.0, -(A + 3.0), A + 2.0],
         [0.0, -A, 2.0 * A + 3.0, -(A + 2.0)],
         [0.0, 0.0, A, -A]], dtype=np.float32)
    return np.stack([
        cubic @ np.array([1.0, d / 4, (d / 4) ** 2, (d / 4) ** 3],
                         dtype=np.float32)
        for d in range(SCALE)
    ])


def make_in_maps(x, kernels):
    """Per-core input dicts with the dtypes the compiled kernel expects."""
    np_io = (np.float16
             if _cfg()["io_dtype"] == "float16" else np.float32)
    ut = _build_ut(np.asarray(kernels, dtype=np.float32)).astype(np_io)
    xp = np.pad(np.asarray(x, dtype=np.float32),
                ((0, 0), (0, 0), (1, 2), (1, 2)), mode="edge").astype(np_io)
    in_maps = []
    for i in range(NCORES):
        shard = np.ascontiguousarray(
            xp[i * IMGS_PER_CORE:(i + 1) * IMGS_PER_CORE].reshape(NIC, HP, HP)
        )
        in_maps.append({"xp": shard, "ut": ut})
    return in_maps


def kernel(x, kernels=None, n_reps: int = 1):
    from concourse.bass_utils import run_bass_kernel_spmd

    if kernels is None:
        kernels = _default_kernels()
    in_maps = make_in_maps(x, kernels)

    nc = get_nc(n_reps)
    res = run_bass_kernel_spmd(nc, in_maps, core_ids=list(range(NCORES)))

    out = np.empty((N, C, HO, WO), dtype=np.float32)
    for i in range(NCORES):
        out[i * IMGS_PER_CORE:(i + 1) * IMGS_PER_CORE] = (
            res.results[i]["out"].astype(np.float32)
            .reshape(IMGS_PER_CORE, C, HO, WO)
        )
    return out
